# revision 24
# baseline (speedup 1.0000x reference)
"""Trainium2 Bass kernel for nn_Attention_23424751632639.

Computation (per (b,h)):  out = tril_strict(rope(Q) @ rope(Q).T / sqrt(N)) @ V
Chunked linear attention (exact reordering of the sums), chunk = 128 rows:
  out_c = QR_c @ M_{c-1}  +  strict_mask(QR_c @ QR_c^T) @ V_c
  M_c   = M_{c-1} + QR_c^T @ V_c          (M = running [64,64] state, PSUM)

Implementation (v3):
  * fp16 everywhere on device; all matmul accumulation stays fp32 in PSUM.
  * RoPE (elementwise) is applied on the host; the device receives QR in both
    natural [t, n] and transposed [n, t] layouts plus V, all fp16, pre-laid
    out per-partition so every DMA moves multi-KB contiguous runs (13 total
    dma_starts).  The scores scale N**-0.5 is folded into the rope tables.
  * Per chunk (4 heads) the PE runs: 4 state matmuls, 4 S blocks + 4 inter
    matmuls (S and inter share the same qrt stationary operand), 4 intra
    matmuls.  All matmul operands sit at partition base 0 (base-64 operands
    fault the device).
  * intra(c) is issued one chunk late so the strict-mask multiply (on
    DVE/ACT/GpSimd) never stalls the PE.
  * PSUM zero-region discipline: one start=True on the first write of each
    2KB region, one stop=True on the last; everything between accumulates.
  * PSUM->SBUF crossings (P-mask, M snapshot, output copy) are statically
    rotated across DVE / ACT / GpSimd.

Sharding: B*H = 32 (b,h) pairs -> 4 per core across 8 cores; no collectives.
"""

import math
import sys

import numpy as np

if "/opt/trn_rl_repo" not in sys.path:
    sys.path.insert(0, "/opt/trn_rl_repo")

B, H, T, N = 2, 16, 4096, 64
THETA = 2.0 ** 16
NCORES = 8
HPC = (B * H) // NCORES   # heads per core
CH = T // 128             # chunks per head (32)
NW = 4                    # windows
CPW = CH // NW            # chunks per window (8)
WCOLS = CPW * HPC * N     # columns per (window, stream) slice (2048)


def build_program():
    import concourse.mybir as mybir
    import concourse.tile as tile
    from concourse import bacc

    f32 = mybir.dt.float32
    f16 = mybir.dt.float16

    nc = bacc.Bacc(None, target_bir_lowering=False)
    # qn: [p, w, s, cw, h, n]; s: 0=qr 1=v       (natural layouts)
    qn = nc.dram_tensor("qn", [128, NW * 2 * WCOLS], f16, kind="ExternalInput")
    # qt: [p(n), w, cw, h, t]                    (transposed rope(Q))
    qt = nc.dram_tensor("qt", [64, NW * 2 * WCOLS], f16, kind="ExternalInput")
    cst = nc.dram_tensor("cst", [128, 512], f16, kind="ExternalInput")
    # o: [p, w, cw, h, n]
    o = nc.dram_tensor("o", [128, NW * WCOLS], f16, kind="ExternalOutput")

    with tile.TileContext(nc) as tc:
        with (
            tc.tile_pool(name="big", bufs=1) as bigp,
            tc.tile_pool(name="mb", bufs=2) as mbp,
            tc.tile_pool(name="psb", bufs=4) as psbp,
            tc.tile_pool(name="tmp", bufs=3) as tmpp,
            tc.tile_pool(name="ost", bufs=3) as ostp,
            tc.tile_pool(name="spps", bufs=3, space="PSUM") as spp,
            tc.tile_pool(name="outps", bufs=3, space="PSUM") as outp,
            tc.tile_pool(name="mps", bufs=1, space="PSUM") as mpp,
        ):
            qn_sb = bigp.tile([128, NW * 2 * WCOLS], f16)
            qt_sb = bigp.tile([64, NW * 2 * WCOLS], f16)
            cst_sb = bigp.tile([128, 512], f16)
            mask4 = cst_sb[:, 0:512]

            qn4 = qn.rearrange("p (w s q) -> p w s q", w=NW, s=2)
            qnsb4 = qn_sb.rearrange("p (w s q) -> p w s q", w=NW, s=2)
            qt4 = qt.rearrange("p (w q) -> p w q", w=NW)
            qtsb4 = qt_sb.rearrange("p (w q) -> p w q", w=NW)

            def dma_qn(w, clo, chi, eng=None):
                a, b = 256 * clo, 256 * chi
                (eng or nc.sync).dma_start(
                    qnsb4[:, w, :, a:b], qn4[:, w, :, a:b])

            def dma_qt(w, clo, chi, eng=None):
                a, b = 512 * clo, 512 * chi
                (eng or nc.sync).dma_start(
                    qtsb4[:, w, a:b], qt4[:, w, a:b])

            mreg = mpp.tile([64, 256], f32, name="mreg")

            # per-chunk records for the 2-chunk-lagged intra
            rec = {}

            def body(c):
                w, cl = c // CPW, c % CPW
                pw, k = cl // 2, cl % 2
                base = w * 2 * WCOLS

                def qr_sl(h):  # [128, 64] natural rope(Q) chunk
                    off = base + 256 * cl + 64 * h
                    return qn_sb[:, off:off + 64]

                def v_sl(h):   # [128, 64] V chunk
                    off = base + WCOLS + 256 * cl + 64 * h
                    return qn_sb[:, off:off + 64]

                def qrt_sl(h):  # [64, 128] transposed rope(Q) chunk
                    off = 2 * w * WCOLS + 512 * cl + 128 * h
                    return qt_sb[:, off:off + 128]

                # state: M_h += QR_c^T V_c   (PSUM accumulate across chunks)
                for h in range(HPC):
                    nc.tensor.matmul(
                        mreg[:, 64 * h:64 * h + 64],
                        qr_sl(h), v_sl(h),
                        start=(c == 0 and h == 0),
                        stop=(c == CH - 1 and h == HPC - 1),
                        skip_group_check=True,
                    )

                # M snapshot for inter of chunk c+1
                mb = None
                if c < CH - 1:
                    mb = mbp.tile([64, 256], f16, tag="mb")
                    if c % 2 == 0:
                        nc.vector.tensor_copy(mb[:], mreg[:])
                    else:
                        nc.scalar.copy(mb[:], mreg[:])

                # output PSUM tile per pair
                if k == 0:
                    op = outp.tile([128, 512], f32, tag="outp")
                else:
                    op = rec[c - 1]["op"]

                # S blocks (+ inter sharing the same stationary operand)
                sp = spp.tile([128, 512], f32, tag="sp")
                for h in range(HPC):
                    qrt_c = qrt_sl(h)
                    nc.tensor.matmul(
                        sp[:, 128 * h:128 * h + 128], qrt_c, qrt_c,
                        start=(h == 0), stop=(h == HPC - 1),
                    )
                    if c > 0:
                        # first write of this pair's outp zero region gets
                        # start=True (inter of even chunk; chunk 1 for pair 0)
                        nc.tensor.matmul(
                            op[:, 256 * k + 64 * h:256 * k + 64 * h + 64],
                            qrt_c, rec[c - 1]["mb"][:, 64 * h:64 * h + 64],
                            start=(h == 0 and (k == 0 or c == 1)),
                            stop=False,
                        )

                # P = S * strict-upper mask  (psum f32 -> sbuf fp16)
                psb = psbp.tile([128, 512], f16, tag="psb")
                r = c % 4
                if r in (1, 3):
                    nc.vector.tensor_mul(psb[:], sp[:], mask4)
                else:
                    tmp = tmpp.tile([128, 512], f16, tag="tmp")
                    nc.scalar.copy(tmp[:], sp[:])
                    if r == 0:
                        nc.vector.tensor_mul(psb[:], tmp[:], mask4)
                    else:
                        nc.gpsimd.tensor_mul(psb[:], tmp[:], mask4)

                # intra lagged by 2 chunks so the mask never stalls the PE
                if c > 1:
                    intra(c - 2)

                rec[c] = {"mb": mb, "psb": psb, "op": op,
                          "v": [v_sl(h) for h in range(HPC)]}
                rec.pop(c - 3, None)

            def intra(c):
                w, cl = c // CPW, c % CPW
                pw, k = cl // 2, cl % 2
                r = rec[c]
                for h in range(HPC):
                    nc.tensor.matmul(
                        r["op"][:, 256 * k + 64 * h:256 * k + 64 * h + 64],
                        r["psb"][:, 128 * h:128 * h + 128], r["v"][h],
                        start=False, stop=(k == 1 and h == HPC - 1),
                    )
                if k == 1:
                    # pair finished: fp16 staging copy + per-pair output DMA
                    ost = ostp.tile([128, 512], f16, tag="ost")
                    if pw % 2 == 0:
                        nc.scalar.copy(ost[:], r["op"][:])
                    else:
                        nc.vector.tensor_copy(ost[:], r["op"][:])
                    off = w * WCOLS + 512 * pw
                    nc.sync.dma_start(o[:, off:off + 512], ost[:])

            # ---- schedule ----
            # prologue: 3-way split per stream, alternating issue queues so
            # compute starts after ~1 chunk of data
            dma_qn(0, 0, 1)
            dma_qt(0, 0, 1, nc.scalar)
            nc.scalar.dma_start(cst_sb[:], cst[:])
            dma_qn(0, 1, 4)
            dma_qt(0, 1, 4, nc.scalar)
            dma_qn(0, 4, CPW)
            dma_qt(0, 4, CPW, nc.scalar)

            for c in range(CH):
                w, cl = c // CPW, c % CPW
                if w < NW - 1:
                    if cl == 1:
                        dma_qt(w + 1, 0, CPW, nc.scalar)
                    elif cl == 2:
                        dma_qn(w + 1, 0, CPW)
                body(c)
            intra(CH - 2)
            intra(CH - 1)

    nc.compile()
    return nc


_CACHE = {}


def _get_program():
    if "nc" not in _CACHE:
        _CACHE["nc"] = build_program()
    return _CACHE["nc"]


def _tables():
    n = np.arange(N, dtype=np.float64)
    tq = np.floor(n / 2.0) * 2.0
    freqs = 1.0 / (THETA ** (tq / N)) / (2.0 * math.pi)
    t = np.arange(T, dtype=np.float64)[:, None]
    ang = ((t * freqs[None, :]) % 1.0) * (2.0 * math.pi)
    scale = float(N) ** -0.25
    cc = (np.cos(ang) * scale).astype(np.float32)
    ss = (np.sin(ang) * scale).astype(np.float32)
    ss[:, 0::2] *= -1.0
    return cc, ss


def make_inputs(Q, V):
    """Full inputs -> list of per-core {'qn','qt','cst'} fp16 host arrays."""
    Q = np.asarray(Q, dtype=np.float32).reshape(NCORES, HPC, T, N)
    V = np.asarray(V, dtype=np.float32).reshape(NCORES, HPC, T, N)
    cc, ss = _tables()
    sq = np.empty_like(Q)
    sq[..., 0::2] = Q[..., 1::2]
    sq[..., 1::2] = Q[..., 0::2]
    qr = (Q * cc + sq * ss).astype(np.float16)  # scaled rope(Q)
    v16 = V.astype(np.float16)

    # natural: [core, h, w, cw, p, n] -> [core, p, w, (cw h n)]
    def nat(x):
        x = x.reshape(NCORES, HPC, NW, CPW, 128, N)
        return np.transpose(x, (0, 4, 2, 3, 1, 5))  # core p w cw h n

    qn_h = np.stack([nat(qr), nat(v16)], axis=3)  # core p w s cw h n
    qn_h = np.ascontiguousarray(qn_h.reshape(NCORES, 128, NW * 2 * WCOLS))

    # transposed: [core, n, w, (cw h t)]
    qt_h = qr.reshape(NCORES, HPC, NW, CPW, 128, N)
    qt_h = np.transpose(qt_h, (0, 5, 2, 3, 1, 4))  # core n w cw h t
    qt_h = np.ascontiguousarray(qt_h.reshape(NCORES, 64, NW * 2 * WCOLS))

    mu = np.triu(np.ones((128, 128), dtype=np.float16), k=1)
    cst = np.ascontiguousarray(np.concatenate([mu] * 4, axis=1))  # [128, 512]
    return [{"qn": qn_h[i], "qt": qt_h[i], "cst": cst}
            for i in range(NCORES)]


def unpack_out(results):
    """list of per-core {'o': [128, NW*WCOLS] fp16} -> [B,H,T,N] f32."""
    o = np.stack([r["o"] for r in results], axis=0)
    o = o.reshape(NCORES, 128, NW, CPW, HPC, N)
    o = np.transpose(o, (0, 4, 2, 3, 1, 5))  # [8, HPC, NW, CPW, 128, N]
    return np.ascontiguousarray(
        o.reshape(B, H, T, N).astype(np.float32))


def kernel(Q, V):
    from concourse.bass_utils import run_bass_kernel_spmd

    nc = _get_program()
    in_maps = make_inputs(Q, V)
    res = run_bass_kernel_spmd(nc, in_maps, core_ids=list(range(NCORES)))
    return unpack_out(res.results)


# revision 25
# speedup vs baseline: 1.1696x; 1.1696x over previous
"""Trainium2 Bass kernel for nn_Attention_23424751632639.

Computation (per (b,h)):  out = tril_strict(rope(Q) @ rope(Q).T / sqrt(N)) @ V
Chunked linear attention (exact reordering of the sums), chunk = 128 rows:
  out_c = QR_c @ M_{c-1}  +  strict_mask(QR_c @ QR_c^T) @ V_c
  M_c   = M_{c-1} + QR_c^T @ V_c          (M = running [64,64] state, PSUM)

Implementation (v3):
  * fp16 everywhere on device; all matmul accumulation stays fp32 in PSUM.
  * RoPE (elementwise) is applied on the host; the device receives QR in both
    natural [t, n] and transposed [n, t] layouts plus V, all fp16, pre-laid
    out per-partition so every DMA moves multi-KB contiguous runs (13 total
    dma_starts).  The scores scale N**-0.5 is folded into the rope tables.
  * Per chunk (4 heads) the PE runs: 4 state matmuls, 4 S blocks + 4 inter
    matmuls (S and inter share the same qrt stationary operand), 4 intra
    matmuls.  All matmul operands sit at partition base 0 (base-64 operands
    fault the device).
  * intra(c) is issued one chunk late so the strict-mask multiply (on
    DVE/ACT/GpSimd) never stalls the PE.
  * PSUM zero-region discipline: one start=True on the first write of each
    2KB region, one stop=True on the last; everything between accumulates.
  * PSUM->SBUF crossings (P-mask, M snapshot, output copy) are statically
    rotated across DVE / ACT / GpSimd.

Sharding: B*H = 32 (b,h) pairs -> 4 per core across 8 cores; no collectives.
"""

import math
import sys

import numpy as np

if "/opt/trn_rl_repo" not in sys.path:
    sys.path.insert(0, "/opt/trn_rl_repo")

B, H, T, N = 2, 16, 4096, 64
THETA = 2.0 ** 16
NCORES = 8
HPC = (B * H) // NCORES   # heads per core
CH = T // 128             # chunks per head (32)
NW = 4                    # windows
CPW = CH // NW            # chunks per window (8)
WCOLS = CPW * HPC * N     # columns per (window, stream) slice (2048)


def build_program():
    import concourse.mybir as mybir
    import concourse.tile as tile
    from concourse import bacc

    f32 = mybir.dt.float32
    f16 = mybir.dt.float16

    nc = bacc.Bacc(None, target_bir_lowering=False)
    # qn: [p, w, s, cw, h, n]; s: 0=qr 1=v       (natural layouts)
    qn = nc.dram_tensor("qn", [128, NW * 2 * WCOLS], f16, kind="ExternalInput")
    # qt: [p(n), w, cw, h, t]                    (transposed rope(Q))
    qt = nc.dram_tensor("qt", [64, NW * 2 * WCOLS], f16, kind="ExternalInput")
    cst = nc.dram_tensor("cst", [128, 512], f16, kind="ExternalInput")
    # o: [p, w, cw, h, n]
    o = nc.dram_tensor("o", [128, NW * WCOLS], f16, kind="ExternalOutput")

    with tile.TileContext(nc) as tc:
        with (
            tc.tile_pool(name="big", bufs=1) as bigp,
            tc.tile_pool(name="mb", bufs=2) as mbp,
            tc.tile_pool(name="psb", bufs=4) as psbp,
            tc.tile_pool(name="tmp", bufs=3) as tmpp,
            tc.tile_pool(name="ost", bufs=3) as ostp,
            tc.tile_pool(name="spps", bufs=3, space="PSUM") as spp,
            tc.tile_pool(name="outps", bufs=3, space="PSUM") as outp,
            tc.tile_pool(name="mps", bufs=1, space="PSUM") as mpp,
        ):
            qn_sb = bigp.tile([128, NW * 2 * WCOLS], f16)
            qt_sb = bigp.tile([64, NW * 2 * WCOLS], f16)
            cst_sb = bigp.tile([128, 512], f16)
            mask4 = cst_sb[:, 0:512]

            qn4 = qn.rearrange("p (w s q) -> p w s q", w=NW, s=2)
            qnsb4 = qn_sb.rearrange("p (w s q) -> p w s q", w=NW, s=2)
            qt4 = qt.rearrange("p (w q) -> p w q", w=NW)
            qtsb4 = qt_sb.rearrange("p (w q) -> p w q", w=NW)

            def dma_qn(w, clo, chi, eng=None):
                a, b = 256 * clo, 256 * chi
                (eng or nc.sync).dma_start(
                    qnsb4[:, w, :, a:b], qn4[:, w, :, a:b])

            def dma_qt(w, clo, chi, eng=None):
                a, b = 512 * clo, 512 * chi
                (eng or nc.sync).dma_start(
                    qtsb4[:, w, a:b], qt4[:, w, a:b])

            mreg = mpp.tile([64, 256], f32, name="mreg")

            # per-chunk records for the 2-chunk-lagged intra
            rec = {}

            def body(c):
                w, cl = c // CPW, c % CPW
                pw, k = cl // 2, cl % 2
                base = w * 2 * WCOLS

                def qr_sl(h):  # [128, 64] natural rope(Q) chunk
                    off = base + 256 * cl + 64 * h
                    return qn_sb[:, off:off + 64]

                def v_sl(h):   # [128, 64] V chunk
                    off = base + WCOLS + 256 * cl + 64 * h
                    return qn_sb[:, off:off + 64]

                def qrt_sl(h):  # [64, 128] transposed rope(Q) chunk
                    off = 2 * w * WCOLS + 512 * cl + 128 * h
                    return qt_sb[:, off:off + 128]

                # state: M_h += QR_c^T V_c   (PSUM accumulate across chunks)
                for h in range(HPC):
                    nc.tensor.matmul(
                        mreg[:, 64 * h:64 * h + 64],
                        qr_sl(h), v_sl(h),
                        start=(c == 0 and h == 0),
                        stop=(c == CH - 1 and h == HPC - 1),
                        skip_group_check=True,
                    )

                # M snapshot for inter of chunk c+1
                mb = None
                if c < CH - 1:
                    mb = mbp.tile([64, 256], f16, tag="mb")
                    if c % 2 == 0:
                        nc.vector.tensor_copy(mb[:], mreg[:])
                    else:
                        nc.scalar.copy(mb[:], mreg[:])

                # output PSUM tile per pair
                if k == 0:
                    op = outp.tile([128, 512], f32, tag="outp")
                else:
                    op = rec[c - 1]["op"]

                # S blocks (+ inter sharing the same stationary operand)
                sp = spp.tile([128, 512], f32, tag="sp")
                for h in range(HPC):
                    qrt_c = qrt_sl(h)
                    nc.tensor.matmul(
                        sp[:, 128 * h:128 * h + 128], qrt_c, qrt_c,
                        start=(h == 0), stop=(h == HPC - 1),
                    )
                    if c > 0:
                        # first write of this pair's outp zero region gets
                        # start=True (inter of even chunk; chunk 1 for pair 0)
                        nc.tensor.matmul(
                            op[:, 256 * k + 64 * h:256 * k + 64 * h + 64],
                            qrt_c, rec[c - 1]["mb"][:, 64 * h:64 * h + 64],
                            start=(h == 0 and (k == 0 or c == 1)),
                            stop=False,
                        )

                # P = S * strict-upper mask  (psum f32 -> sbuf fp16)
                psb = psbp.tile([128, 512], f16, tag="psb")
                r = c % 4
                if r in (1, 3):
                    nc.vector.tensor_mul(psb[:], sp[:], mask4)
                else:
                    tmp = tmpp.tile([128, 512], f16, tag="tmp")
                    nc.scalar.copy(tmp[:], sp[:])
                    if r == 0:
                        nc.vector.tensor_mul(psb[:], tmp[:], mask4)
                    else:
                        nc.gpsimd.tensor_mul(psb[:], tmp[:], mask4)

                # intra lagged by 2 chunks so the mask never stalls the PE
                if c > 1:
                    intra(c - 2)

                rec[c] = {"mb": mb, "psb": psb, "op": op,
                          "v": [v_sl(h) for h in range(HPC)]}
                rec.pop(c - 3, None)

            def intra(c):
                w, cl = c // CPW, c % CPW
                pw, k = cl // 2, cl % 2
                r = rec[c]
                for h in range(HPC):
                    nc.tensor.matmul(
                        r["op"][:, 256 * k + 64 * h:256 * k + 64 * h + 64],
                        r["psb"][:, 128 * h:128 * h + 128], r["v"][h],
                        start=False, stop=(k == 1 and h == HPC - 1),
                    )
                if k == 1:
                    # pair finished: fp16 staging copy + per-pair output DMA
                    ost = ostp.tile([128, 512], f16, tag="ost")
                    if pw % 2 == 0:
                        nc.scalar.copy(ost[:], r["op"][:])
                    else:
                        nc.vector.tensor_copy(ost[:], r["op"][:])
                    off = w * WCOLS + 512 * pw
                    nc.sync.dma_start(o[:, off:off + 512], ost[:])

            # ---- schedule ----
            # prologue: 3-way split per stream, alternating issue queues so
            # compute starts after ~1 chunk of data
            dma_qn(0, 0, 1)
            dma_qt(0, 0, 1, nc.scalar)
            nc.scalar.dma_start(cst_sb[:], cst[:])
            dma_qn(0, 1, 4)
            dma_qt(0, 1, 4, nc.scalar)
            dma_qn(0, 4, CPW)
            dma_qt(0, 4, CPW, nc.scalar)

            for c in range(CH):
                w, cl = c // CPW, c % CPW
                if w < NW - 1:
                    if cl == 0:
                        dma_qt(w + 1, 0, CPW)
                    elif cl == 1:
                        dma_qn(w + 1, 0, CPW)
                body(c)
            intra(CH - 2)
            intra(CH - 1)

    nc.compile()
    return nc


_CACHE = {}


def _get_program():
    if "nc" not in _CACHE:
        _CACHE["nc"] = build_program()
    return _CACHE["nc"]


def _tables():
    n = np.arange(N, dtype=np.float64)
    tq = np.floor(n / 2.0) * 2.0
    freqs = 1.0 / (THETA ** (tq / N)) / (2.0 * math.pi)
    t = np.arange(T, dtype=np.float64)[:, None]
    ang = ((t * freqs[None, :]) % 1.0) * (2.0 * math.pi)
    scale = float(N) ** -0.25
    cc = (np.cos(ang) * scale).astype(np.float32)
    ss = (np.sin(ang) * scale).astype(np.float32)
    ss[:, 0::2] *= -1.0
    return cc, ss


def make_inputs(Q, V):
    """Full inputs -> list of per-core {'qn','qt','cst'} fp16 host arrays."""
    Q = np.asarray(Q, dtype=np.float32).reshape(NCORES, HPC, T, N)
    V = np.asarray(V, dtype=np.float32).reshape(NCORES, HPC, T, N)
    cc, ss = _tables()
    sq = np.empty_like(Q)
    sq[..., 0::2] = Q[..., 1::2]
    sq[..., 1::2] = Q[..., 0::2]
    qr = (Q * cc + sq * ss).astype(np.float16)  # scaled rope(Q)
    v16 = V.astype(np.float16)

    # natural: [core, h, w, cw, p, n] -> [core, p, w, (cw h n)]
    def nat(x):
        x = x.reshape(NCORES, HPC, NW, CPW, 128, N)
        return np.transpose(x, (0, 4, 2, 3, 1, 5))  # core p w cw h n

    qn_h = np.stack([nat(qr), nat(v16)], axis=3)  # core p w s cw h n
    qn_h = np.ascontiguousarray(qn_h.reshape(NCORES, 128, NW * 2 * WCOLS))

    # transposed: [core, n, w, (cw h t)]
    qt_h = qr.reshape(NCORES, HPC, NW, CPW, 128, N)
    qt_h = np.transpose(qt_h, (0, 5, 2, 3, 1, 4))  # core n w cw h t
    qt_h = np.ascontiguousarray(qt_h.reshape(NCORES, 64, NW * 2 * WCOLS))

    mu = np.triu(np.ones((128, 128), dtype=np.float16), k=1)
    cst = np.ascontiguousarray(np.concatenate([mu] * 4, axis=1))  # [128, 512]
    return [{"qn": qn_h[i], "qt": qt_h[i], "cst": cst}
            for i in range(NCORES)]


def unpack_out(results):
    """list of per-core {'o': [128, NW*WCOLS] fp16} -> [B,H,T,N] f32."""
    o = np.stack([r["o"] for r in results], axis=0)
    o = o.reshape(NCORES, 128, NW, CPW, HPC, N)
    o = np.transpose(o, (0, 4, 2, 3, 1, 5))  # [8, HPC, NW, CPW, 128, N]
    return np.ascontiguousarray(
        o.reshape(B, H, T, N).astype(np.float32))


def kernel(Q, V):
    from concourse.bass_utils import run_bass_kernel_spmd

    nc = _get_program()
    in_maps = make_inputs(Q, V)
    res = run_bass_kernel_spmd(nc, in_maps, core_ids=list(range(NCORES)))
    return unpack_out(res.results)


# revision 26
# speedup vs baseline: 1.1909x; 1.0182x over previous
"""Trainium2 Bass kernel for nn_Attention_23424751632639.

Computation (per (b,h)):  out = tril_strict(rope(Q) @ rope(Q).T / sqrt(N)) @ V
Chunked linear attention (exact reordering of the sums), chunk = 128 rows:
  out_c = QR_c @ M_{c-1}  +  strict_mask(QR_c @ QR_c^T) @ V_c
  M_c   = M_{c-1} + QR_c^T @ V_c          (M = running [64,64] state, PSUM)

Implementation (v3):
  * fp16 everywhere on device; all matmul accumulation stays fp32 in PSUM.
  * RoPE (elementwise) is applied on the host; the device receives QR in both
    natural [t, n] and transposed [n, t] layouts plus V, all fp16, pre-laid
    out per-partition so every DMA moves multi-KB contiguous runs (13 total
    dma_starts).  The scores scale N**-0.5 is folded into the rope tables.
  * Per chunk (4 heads) the PE runs: 4 state matmuls, 4 S blocks + 4 inter
    matmuls (S and inter share the same qrt stationary operand), 4 intra
    matmuls.  All matmul operands sit at partition base 0 (base-64 operands
    fault the device).
  * intra(c) is issued one chunk late so the strict-mask multiply (on
    DVE/ACT/GpSimd) never stalls the PE.
  * PSUM zero-region discipline: one start=True on the first write of each
    2KB region, one stop=True on the last; everything between accumulates.
  * PSUM->SBUF crossings (P-mask, M snapshot, output copy) are statically
    rotated across DVE / ACT / GpSimd.

Sharding: B*H = 32 (b,h) pairs -> 4 per core across 8 cores; no collectives.
"""

import math
import sys

import numpy as np

if "/opt/trn_rl_repo" not in sys.path:
    sys.path.insert(0, "/opt/trn_rl_repo")

B, H, T, N = 2, 16, 4096, 64
THETA = 2.0 ** 16
NCORES = 8
HPC = (B * H) // NCORES   # heads per core
CH = T // 128             # chunks per head (32)
NW = 4                    # windows
CPW = CH // NW            # chunks per window (8)
WCOLS = CPW * HPC * N     # columns per (window, stream) slice (2048)


def build_program():
    import concourse.mybir as mybir
    import concourse.tile as tile
    from concourse import bacc

    f32 = mybir.dt.float32
    f16 = mybir.dt.float16

    nc = bacc.Bacc(None, target_bir_lowering=False)
    # qn: [p, w, s, cw, h, n]; s: 0=qr 1=v       (natural layouts)
    qn = nc.dram_tensor("qn", [128, NW * 2 * WCOLS], f16, kind="ExternalInput")
    # qt: [p(n), w, cw, h, t]                    (transposed rope(Q))
    qt = nc.dram_tensor("qt", [64, NW * 2 * WCOLS], f16, kind="ExternalInput")
    cst = nc.dram_tensor("cst", [128, 512], f16, kind="ExternalInput")
    # o: [p, w, cw, h, n]
    o = nc.dram_tensor("o", [128, NW * WCOLS], f16, kind="ExternalOutput")

    with tile.TileContext(nc) as tc:
        with (
            tc.tile_pool(name="big", bufs=1) as bigp,
            tc.tile_pool(name="mb", bufs=2) as mbp,
            tc.tile_pool(name="psb", bufs=4) as psbp,
            tc.tile_pool(name="tmp", bufs=3) as tmpp,
            tc.tile_pool(name="ost", bufs=3) as ostp,
            tc.tile_pool(name="spps", bufs=3, space="PSUM") as spp,
            tc.tile_pool(name="outps", bufs=3, space="PSUM") as outp,
            tc.tile_pool(name="mps", bufs=1, space="PSUM") as mpp,
        ):
            qn_sb = bigp.tile([128, NW * 2 * WCOLS], f16)
            qt_sb = bigp.tile([64, NW * 2 * WCOLS], f16)
            cst_sb = bigp.tile([128, 512], f16)
            mask4 = cst_sb[:, 0:512]

            qn4 = qn.rearrange("p (w s q) -> p w s q", w=NW, s=2)
            qnsb4 = qn_sb.rearrange("p (w s q) -> p w s q", w=NW, s=2)
            qt4 = qt.rearrange("p (w q) -> p w q", w=NW)
            qtsb4 = qt_sb.rearrange("p (w q) -> p w q", w=NW)

            def dma_qn(w, clo, chi, eng=None):
                a, b = 256 * clo, 256 * chi
                (eng or nc.sync).dma_start(
                    qnsb4[:, w, :, a:b], qn4[:, w, :, a:b])

            def dma_qt(w, clo, chi, eng=None):
                a, b = 512 * clo, 512 * chi
                (eng or nc.sync).dma_start(
                    qtsb4[:, w, a:b], qt4[:, w, a:b])

            mreg = mpp.tile([64, 256], f32, name="mreg")

            # per-chunk records for the 2-chunk-lagged intra
            rec = {}

            def body(c):
                w, cl = c // CPW, c % CPW
                pw, k = cl // 2, cl % 2
                base = w * 2 * WCOLS

                def qr_sl(h):  # [128, 64] natural rope(Q) chunk
                    off = base + 256 * cl + 64 * h
                    return qn_sb[:, off:off + 64]

                def v_sl(h):   # [128, 64] V chunk
                    off = base + WCOLS + 256 * cl + 64 * h
                    return qn_sb[:, off:off + 64]

                def qrt_sl(h):  # [64, 128] transposed rope(Q) chunk
                    off = 2 * w * WCOLS + 512 * cl + 128 * h
                    return qt_sb[:, off:off + 128]

                # state: M_h += QR_c^T V_c   (PSUM accumulate across chunks)
                for h in range(HPC):
                    nc.tensor.matmul(
                        mreg[:, 64 * h:64 * h + 64],
                        qr_sl(h), v_sl(h),
                        start=(c == 0 and h == 0),
                        stop=(c == CH - 1 and h == HPC - 1),
                        skip_group_check=True,
                    )

                # M snapshot for inter of chunk c+1
                mb = None
                if c < CH - 1:
                    mb = mbp.tile([64, 256], f16, tag="mb")
                    if c % 2 == 0:
                        nc.vector.tensor_copy(mb[:], mreg[:])
                    else:
                        nc.scalar.copy(mb[:], mreg[:])

                # output PSUM tile per pair
                if k == 0:
                    op = outp.tile([128, 512], f32, tag="outp")
                else:
                    op = rec[c - 1]["op"]

                # S blocks (+ inter sharing the same stationary operand)
                sp = spp.tile([128, 512], f32, tag="sp")
                for h in range(HPC):
                    qrt_c = qrt_sl(h)
                    nc.tensor.matmul(
                        sp[:, 128 * h:128 * h + 128], qrt_c, qrt_c,
                        start=(h == 0), stop=(h == HPC - 1),
                    )
                    if c > 0:
                        # first write of this pair's outp zero region gets
                        # start=True (inter of even chunk; chunk 1 for pair 0)
                        nc.tensor.matmul(
                            op[:, 256 * k + 64 * h:256 * k + 64 * h + 64],
                            qrt_c, rec[c - 1]["mb"][:, 64 * h:64 * h + 64],
                            start=(h == 0 and (k == 0 or c == 1)),
                            stop=False,
                        )

                # P = S * strict-upper mask  (psum f32 -> sbuf fp16)
                psb = psbp.tile([128, 512], f16, tag="psb")
                r = c % 4
                if r in (1, 3):
                    nc.vector.tensor_mul(psb[:], sp[:], mask4)
                else:
                    tmp = tmpp.tile([128, 512], f16, tag="tmp")
                    nc.scalar.copy(tmp[:], sp[:])
                    if r == 0:
                        nc.vector.tensor_mul(psb[:], tmp[:], mask4)
                    else:
                        nc.gpsimd.tensor_mul(psb[:], tmp[:], mask4)

                # intra lagged by 2 chunks so the mask never stalls the PE
                if c > 1:
                    intra(c - 2)

                rec[c] = {"mb": mb, "psb": psb, "op": op,
                          "v": [v_sl(h) for h in range(HPC)]}
                rec.pop(c - 3, None)

            def intra(c):
                w, cl = c // CPW, c % CPW
                pw, k = cl // 2, cl % 2
                r = rec[c]
                for h in range(HPC):
                    nc.tensor.matmul(
                        r["op"][:, 256 * k + 64 * h:256 * k + 64 * h + 64],
                        r["psb"][:, 128 * h:128 * h + 128], r["v"][h],
                        start=False, stop=(k == 1 and h == HPC - 1),
                    )
                if k == 1:
                    # pair finished: fp16 staging copy + per-pair output DMA
                    ost = ostp.tile([128, 512], f16, tag="ost")
                    if pw % 2 == 0:
                        nc.scalar.copy(ost[:], r["op"][:])
                    else:
                        nc.vector.tensor_copy(ost[:], r["op"][:])
                    off = w * WCOLS + 512 * pw
                    nc.sync.dma_start(o[:, off:off + 512], ost[:])

            # ---- schedule ----
            # prologue: 3-way split per stream, alternating issue queues so
            # compute starts after ~1 chunk of data
            dma_qn(0, 0, 1)
            dma_qt(0, 0, 1, nc.scalar)
            dma_qn(0, 1, 2)
            dma_qt(0, 1, 2, nc.scalar)
            nc.scalar.dma_start(cst_sb[:], cst[:])
            dma_qn(0, 2, 4)
            dma_qt(0, 2, 4, nc.scalar)
            dma_qn(0, 4, CPW)
            dma_qt(0, 4, CPW, nc.scalar)

            for c in range(CH):
                w, cl = c // CPW, c % CPW
                if w < NW - 1:
                    if cl == 0:
                        dma_qt(w + 1, 0, CPW)
                    elif cl == 1:
                        dma_qn(w + 1, 0, CPW)
                body(c)
            intra(CH - 2)
            intra(CH - 1)

    nc.compile()
    return nc


_CACHE = {}


def _get_program():
    if "nc" not in _CACHE:
        _CACHE["nc"] = build_program()
    return _CACHE["nc"]


def _tables():
    n = np.arange(N, dtype=np.float64)
    tq = np.floor(n / 2.0) * 2.0
    freqs = 1.0 / (THETA ** (tq / N)) / (2.0 * math.pi)
    t = np.arange(T, dtype=np.float64)[:, None]
    ang = ((t * freqs[None, :]) % 1.0) * (2.0 * math.pi)
    scale = float(N) ** -0.25
    cc = (np.cos(ang) * scale).astype(np.float32)
    ss = (np.sin(ang) * scale).astype(np.float32)
    ss[:, 0::2] *= -1.0
    return cc, ss


def make_inputs(Q, V):
    """Full inputs -> list of per-core {'qn','qt','cst'} fp16 host arrays."""
    Q = np.asarray(Q, dtype=np.float32).reshape(NCORES, HPC, T, N)
    V = np.asarray(V, dtype=np.float32).reshape(NCORES, HPC, T, N)
    cc, ss = _tables()
    sq = np.empty_like(Q)
    sq[..., 0::2] = Q[..., 1::2]
    sq[..., 1::2] = Q[..., 0::2]
    qr = (Q * cc + sq * ss).astype(np.float16)  # scaled rope(Q)
    v16 = V.astype(np.float16)

    # natural: [core, h, w, cw, p, n] -> [core, p, w, (cw h n)]
    def nat(x):
        x = x.reshape(NCORES, HPC, NW, CPW, 128, N)
        return np.transpose(x, (0, 4, 2, 3, 1, 5))  # core p w cw h n

    qn_h = np.stack([nat(qr), nat(v16)], axis=3)  # core p w s cw h n
    qn_h = np.ascontiguousarray(qn_h.reshape(NCORES, 128, NW * 2 * WCOLS))

    # transposed: [core, n, w, (cw h t)]
    qt_h = qr.reshape(NCORES, HPC, NW, CPW, 128, N)
    qt_h = np.transpose(qt_h, (0, 5, 2, 3, 1, 4))  # core n w cw h t
    qt_h = np.ascontiguousarray(qt_h.reshape(NCORES, 64, NW * 2 * WCOLS))

    mu = np.triu(np.ones((128, 128), dtype=np.float16), k=1)
    cst = np.ascontiguousarray(np.concatenate([mu] * 4, axis=1))  # [128, 512]
    return [{"qn": qn_h[i], "qt": qt_h[i], "cst": cst}
            for i in range(NCORES)]


def unpack_out(results):
    """list of per-core {'o': [128, NW*WCOLS] fp16} -> [B,H,T,N] f32."""
    o = np.stack([r["o"] for r in results], axis=0)
    o = o.reshape(NCORES, 128, NW, CPW, HPC, N)
    o = np.transpose(o, (0, 4, 2, 3, 1, 5))  # [8, HPC, NW, CPW, 128, N]
    return np.ascontiguousarray(
        o.reshape(B, H, T, N).astype(np.float32))


def kernel(Q, V):
    from concourse.bass_utils import run_bass_kernel_spmd

    nc = _get_program()
    in_maps = make_inputs(Q, V)
    res = run_bass_kernel_spmd(nc, in_maps, core_ids=list(range(NCORES)))
    return unpack_out(res.results)
